# revision 1
# baseline (speedup 1.0000x reference)
"""CurricularFace loss kernel for 8 Trainium2 NeuronCores.

Strategy (tensor-parallel classifier, per the sharding hint):
  - Shard the class dimension: core c owns logits[:, c*12500:(c+1)*12500].
  - Host precomputes the per-row (512,) quantities derived from the label
    gather (target_logit, cos_theta_m, final_target_logit) in float32 with
    the exact op sequence of the reference, so the hard-example mask
    compare on device is bit-exact.
  - Device pass 1: stream+clip the shard (DVE tensor_scalar dual-op,
    2x mode), keep most clipped tiles resident in SBUF, accumulate the
    global sum via ones-matmul partition reduction on the idle PE.
  - One scalar AllReduce across the 8 cores gives the EMA statistic t_new.
  - Device pass 2: out = cos * (64 + 64*m*(ctm + t' - 1) + 64*relu(cos-ctm))
    with m = (cos > ctm), computed as:
      rp  = ACT Relu(64*cos - 64*ctm)          (scalar engine, free affine)
      m1  = TS (cos is_gt ctm) * k64           (DVE 2x dual-op)
      t1  = STT (m1 + 64) + rp                 (DVE scalar_tensor_tensor)
      out = STT (t1 bypass) * cos              (DVE scalar_tensor_tensor)
    For non-hard elements this reduces to exactly 64*cos (bit-exact with
    the reference); hard elements differ only by ~ulp-level rounding.
  - Host applies the label-column scatter (64*final_target_logit) while
    reassembling the full (512, 100000) output.
"""

import math
import os
import sys

import numpy as np

if "/opt/trn_rl_repo" not in sys.path:
    sys.path.insert(0, "/opt/trn_rl_repo")

import concourse.bacc as bacc
import concourse.mybir as mybir
import concourse.tile as tile
from concourse import bass_utils

# Problem constants (hardcoded per contract).
B, C = 512, 100000
N_CORES = 8
COLS = C // N_CORES          # 12500 columns per core
FT = 2500                    # tile free dim
NCH = B // 128               # 4 row chunks of 128 partitions
NJT = COLS // FT             # 5 column tiles per chunk
NT = NCH * NJT               # 20 tiles per core
R_TILES = int(os.environ.get("KR_RES", "12"))   # clipped tiles kept resident
XS_BUFS = int(os.environ.get("KR_XSBUFS", "3"))  # streaming slot ring depth
MMQ = 500                    # matmul free-dim chunk for the PE row-sum

MARGIN = 0.5
S = 64.0
COS_M = math.cos(MARGIN)
SIN_M = math.sin(MARGIN)
THRESHOLD = math.cos(math.pi - MARGIN)
MM = math.sin(math.pi - MARGIN) * MARGIN

AOT = mybir.AluOpType
AFT = mybir.ActivationFunctionType
F32 = mybir.dt.float32

_nc_cache = None


def _build_nc():
    nc = bacc.Bacc("TRN2", num_devices=N_CORES)
    x = nc.dram_tensor("x", [B, COLS], F32, kind="ExternalInput")
    ctm_in = nc.dram_tensor("ctm", [128, NCH], F32, kind="ExternalInput")
    nctm64_in = nc.dram_tensor("nctm64", [128, NCH], F32, kind="ExternalInput")
    cst_in = nc.dram_tensor("cst", [1, 2], F32, kind="ExternalInput")
    y = nc.dram_tensor("y", [B, COLS], F32, kind="ExternalOutput")

    tiles = [(r, j) for r in range(NCH) for j in range(NJT)]

    with tile.TileContext(nc) as tc:
        with (
            tc.tile_pool(name="small", bufs=1) as sp,
            tc.tile_pool(name="res", bufs=1) as rp_pool,
            tc.tile_pool(name="work", bufs=1) as wp,
            tc.tile_pool(name="psum", bufs=1, space="PSUM") as pp,
            tc.tile_pool(name="dram", bufs=1, space="DRAM") as dp,
        ):
            ctm_sb = sp.tile([128, NCH], F32)
            nctm64_sb = sp.tile([128, NCH], F32)
            cst_sb = sp.tile([1, 2], F32)
            ones = sp.tile([128, 1], F32)
            sums = sp.tile([128, NT // 2], F32)
            nc.sync.dma_start(ctm_sb[:], ctm_in[:])
            nc.sync.dma_start(nctm64_sb[:], nctm64_in[:])
            nc.sync.dma_start(cst_sb[:], cst_in[:])
            nc.vector.memset(ones[:], 1.0)

            ps = pp.tile([1, MMQ], F32)

            # ---- pass 1: clip (in-place) + global-sum partials ---------
            # Even tiles feed the idle PE (ones-matmul partition reduce),
            # odd tiles use DVE tensor_reduce; both stay under the DMA-in
            # time so pass 1 is memory-bound.  Streamed tiles are spread
            # through the pass so their slot-ring waits hide under the
            # resident loads; the last XS_BUFS streamed tiles survive in
            # the ring and are reused by pass 2 with no re-read.
            n_stream = NT - R_TILES
            stride = NT / max(n_stream, 1)
            streamed = sorted({min(NT - 1, int((i + 1) * stride) - 1)
                               for i in range(n_stream)})
            if len(streamed) < n_stream:
                extra = [t for t in range(NT) if t not in streamed]
                streamed = sorted(streamed +
                                  extra[:n_stream - len(streamed)])
            res_tiles = {}
            ring_tiles = {}
            nmm = FT // MMQ
            for t, (r, j) in enumerate(tiles):
                rs, cs = r * 128, j * FT
                if t in streamed:
                    xt = wp.tile([128, FT], F32, tag="xs", bufs=XS_BUFS,
                                 name=f"xs{t}")
                    ring_tiles[t] = xt
                else:
                    xt = rp_pool.tile([128, FT], F32, tag=f"xr{t}", bufs=1,
                                      name=f"xr{t}")
                    res_tiles[t] = xt
                nc.sync.dma_start(xt[:], x[rs:rs + 128, cs:cs + FT])
                nc.vector.tensor_scalar(xt[:], xt[:], -1.0, 1.0,
                                        AOT.max, AOT.min)
                if t % 2 == 0:
                    for q in range(nmm):
                        nc.tensor.matmul(ps[:], ones[:],
                                         xt[:, q * MMQ:(q + 1) * MMQ],
                                         start=(t == 0 and q == 0),
                                         stop=False)
                else:
                    nc.vector.tensor_reduce(sums[:, t // 2:t // 2 + 1],
                                            xt[:], mybir.AxisListType.X,
                                            AOT.add)

            # Last XS_BUFS streamed tiles stay valid in the slot ring; the
            # rest are re-read inline during pass 2, post-AllReduce, so
            # they never delay the loads feeding the global sum.
            keep = set(streamed[-XS_BUFS:])
            p2_streamed = {t: ring_tiles[t] for t in keep}
            reread = set(t for t in streamed if t not in keep)

            # ---- scalar chain: total -> AllReduce -> k64 ---------------
            # Fold the DVE-side partials into the same PSUM accumulation
            # group via one more ones-matmul, then one reduce drains it.
            rowsum = sp.tile([128, 1], F32)
            nc.vector.tensor_reduce(rowsum[:], sums[:], mybir.AxisListType.X,
                                    AOT.add)
            nc.tensor.matmul(ps[:, 0:1], ones[:], rowsum[:], start=False,
                             stop=True)
            tot_sb = sp.tile([1, 1], F32)
            nc.vector.tensor_reduce(tot_sb[:], ps[:], mybir.AxisListType.X,
                                    AOT.add)
            cc_in = dp.tile([1, 1], F32)
            cc_out = dp.tile([1, 1], F32, addr_space="Shared")
            nc.sync.dma_start(cc_in[:], tot_sb[:])
            nc.gpsimd.collective_compute(
                "AllReduce", AOT.add,
                replica_groups=[list(range(N_CORES))],
                ins=[cc_in.opt()], outs=[cc_out.opt()],
            )
            tot2 = sp.tile([1, 1], F32)
            nc.sync.dma_start(tot2[:], cc_out[:])
            # t' - 1 = tot * c0 + (0.99*t0 - 1)
            tpm1 = sp.tile([1, 1], F32)
            nc.vector.tensor_scalar(tpm1[:], tot2[:], cst_sb[0:1, 0:1],
                                    cst_sb[0:1, 1:2], AOT.mult, AOT.add)
            tpb = sp.tile([128, 1], F32)
            nc.gpsimd.partition_broadcast(tpb[:], tpm1[:])
            # k64 = 64 * (ctm + (t' - 1))
            k64 = sp.tile([128, NCH], F32)
            nc.vector.tensor_scalar(k64[:], ctm_sb[:], tpb[:, 0:1], 64.0,
                                    AOT.add, AOT.mult)

            # ---- pass 2: elementwise update + writeback ----------------
            # Ring tiles first (processing them frees xs slots for the
            # re-read DMAs), then re-reads interleaved between resident
            # tiles so every re-read load hides under resident compute.
            resident_order = sorted(res_tiles)
            p2_order = sorted(keep)
            ri = 0
            for t in sorted(reread):
                p2_order.append(t)
                if ri < len(resident_order):
                    p2_order.append(resident_order[ri])
                    ri += 1
            p2_order += resident_order[ri:]
            for t in p2_order:
                r, j = tiles[t]
                rs, cs = r * 128, j * FT
                if t in reread:
                    ct = wp.tile([128, FT], F32, tag="xs", bufs=XS_BUFS,
                                 name=f"cs{t}")
                    nc.sync.dma_start(ct[:], x[rs:rs + 128, cs:cs + FT])
                    nc.vector.tensor_scalar(ct[:], ct[:], -1.0, 1.0,
                                            AOT.max, AOT.min)
                    p2_streamed[t] = ct
                ct = p2_streamed[t] if t in p2_streamed else res_tiles[t]
                rp = wp.tile([128, FT], F32, tag="rp", bufs=2, name=f"rp{t}")
                nc.scalar.activation(rp[:], ct[:], AFT.Relu,
                                     bias=nctm64_sb[:, r:r + 1], scale=64.0)
                m1 = wp.tile([128, FT], F32, tag="m1", bufs=3, name=f"m1{t}")
                nc.vector.tensor_scalar(m1[:], ct[:], ctm_sb[:, r:r + 1],
                                        k64[:, r:r + 1], AOT.is_gt, AOT.mult)
                nc.vector.scalar_tensor_tensor(m1[:], m1[:], 64.0, rp[:],
                                               AOT.add, AOT.add)
                nc.vector.scalar_tensor_tensor(m1[:], m1[:], 0.0, ct[:],
                                               AOT.bypass, AOT.mult)
                nc.sync.dma_start(y[rs:rs + 128, cs:cs + FT], m1[:])

    nc.compile()
    return nc


def _get_nc():
    global _nc_cache
    if _nc_cache is None:
        _nc_cache = _build_nc()
    return _nc_cache


def _host_prep(logits, labels, t):
    f32 = np.float32
    labels_i = np.asarray(labels).astype(np.int32)
    valid = labels_i >= 0
    lab = np.where(valid, labels_i, 0)
    rows = np.arange(B)
    tgt = np.ascontiguousarray(logits[rows, lab], dtype=np.float32)
    tl = np.clip(tgt, f32(-1.0), f32(1.0))
    sin = np.sqrt(f32(1.0) - tl * tl)
    ctm = tl * f32(COS_M) - sin * f32(SIN_M)
    ftl = np.where(tl > f32(THRESHOLD), ctm, tl - f32(MM)).astype(np.float32)
    ctm_eff = np.where(valid, ctm, f32(2.0)).astype(np.float32)

    ctm_t = np.ascontiguousarray(ctm_eff.reshape(NCH, 128).T)
    nctm64_t = np.ascontiguousarray((f32(-64.0) * ctm_eff).reshape(NCH, 128).T)

    t0 = f32(np.asarray(t).reshape(-1)[0])
    n_valid = f32(valid.sum())
    c0 = f32(0.01) / (n_valid * f32(C))
    c99tm1 = f32(0.99) * t0 - f32(1.0)
    cst = np.array([[c0, c99tm1]], dtype=np.float32)
    return valid, lab, rows, ftl, ctm_t, nctm64_t, cst


def run(inputs, trace=False):
    logits = np.asarray(inputs["logits"], dtype=np.float32)
    labels = inputs["labels"]
    t = inputs["t"]
    valid, lab, rows, ftl, ctm_t, nctm64_t, cst = _host_prep(logits, labels, t)

    in_maps = []
    for c in range(N_CORES):
        in_maps.append({
            "x": np.ascontiguousarray(logits[:, c * COLS:(c + 1) * COLS]),
            "ctm": ctm_t,
            "nctm64": nctm64_t,
            "cst": cst,
        })
    nc = _get_nc()
    res = bass_utils.run_bass_kernel_spmd(
        nc, in_maps, core_ids=list(range(N_CORES)), trace=trace)
    out = np.concatenate([res.results[c]["y"] for c in range(N_CORES)], axis=1)
    sval = np.float32(S) * ftl
    out[rows[valid], lab[valid]] = sval[valid]
    return out, res


def kernel(**inputs):
    out, _ = run(inputs, trace=False)
    return out

